# revision 45
# baseline (speedup 1.0000x reference)
"""CondMlp Trainium2 kernel.

Math (reference):
    xp = x @ W_pre + b_pre                 # [B, NI, DH]
    c  = query @ W_emb + b_emb             # [B, NO, DH]
    A  = xp @ W1[:DH] + b1                 # [B, NI, DH]   (host precompute, tiny)
    C2 = c @ W1[DH:]                       # [B, NO, DH]   (host precompute, tiny)
    h[b,i,o,:] = A[b,i,:] + C2[b,o,:]
    out[b,i,o,:] = gelu(h) @ W2 + b2       # [B, NI, NO, DOUT]

The heavy part (device): for every (b,i) row, gelu of a [NO, DH] tile followed
by a [NO, DH] @ [DH, DOUT] matmul, and 256 MB of output writes.

Sharding: 8 cores, core k handles batch b = k//2, NI-half h = k%2 (128 rows).

Device layout trick: work in transposed [dh, o] layout so the broadcast add
A[b,i,:] + C2[b,o,:] becomes a per-partition scalar add (DVE tensor_scalar),
and the second matmul uses G.T slices as the stationary operand:
    psum[o_chunk, dout] += G.T[dh_chunk, o_chunk].T @ W2[dh_chunk, :]
which yields output tiles in [o, dout] layout = contiguous HBM rows.
"""

import numpy as np
import ml_dtypes

import concourse.bass as bass
import concourse.bacc as bacc
import concourse.mybir as mybir
from concourse.tile import TileContext
from concourse.bass_utils import run_bass_kernel_spmd

B, NI, NO = 4, 256, 256
DIN, DQ, DH, DOUT = 256, 256, 256, 256
NCORES = 8
RPC = (B * NI) // NCORES    # rows per core = 128
R = 8                       # rows per group
NG = RPC // R               # 16 groups
F32 = mybir.dt.float32
BF16 = mybir.dt.bfloat16

_nc_cache = None


def build_nc():
    # Bacc (not raw Bass): its finalize() runs generate_event_semaphores,
    # which splits multi-sem waits to satisfy the 1-wait-per-instruction
    # TPB ISA constraint (walrus "Too many sync wait commands" otherwise).
    nc = bacc.Bacc()

    # C2.T in bf16: it is the streamed operand of the DVE tensor_scalar adds,
    # and 16-bit streams hit the 2x packed perf mode (fp32 streams at 1x).
    # A.T stays fp32 (bass requires a float32 scalar operand); the add's fp32
    # output keeps the gelu input near-exact.
    c_t = nc.declare_dram_parameter("c_t", [DH, NO], BF16, isOutput=False)
    a_t = nc.declare_dram_parameter("a_t", [DH, RPC], F32, isOutput=False)
    w2 = nc.declare_dram_parameter("w2", [DH, DOUT], BF16, isOutput=False)       # [dh, dout]
    # Flat per-group output: OUT[g, p, (r, c, d)]; host reassembles. Gives the
    # store DMA 16 KiB contiguous runs per partition instead of 1 KiB.
    out = nc.declare_dram_parameter("out", [NG, 128, R * 512], F32, isOutput=True)

    gelu = mybir.ActivationFunctionType.Gelu

    with TileContext(nc) as tc:
        with (
            tc.tile_pool(name="const", bufs=1) as cpool,
            tc.tile_pool(name="h", bufs=3) as hpool,
            tc.tile_pool(name="g", bufs=3) as gpool,
            tc.tile_pool(name="ps", bufs=4, space="PSUM") as pspool,
            tc.tile_pool(name="ostage", bufs=3) as opool,
        ):
            # Constant tiles, loaded once. dh is split into two partition chunks.
            ct = []
            at = []
            w2t = []
            # Spread the six constant loads over two DMA rings so they land in
            # ~2us instead of ~6 serialized (they gate the whole ramp).
            for ch in range(2):
                t = cpool.tile([128, NO], BF16, tag=f"ct{ch}")
                nc.sync.dma_start(out=t[:, :], in_=c_t[ch * 128:(ch + 1) * 128, :])
                ct.append(t)
                t = cpool.tile([128, RPC], F32, tag=f"at{ch}")
                nc.sync.dma_start(out=t[:, :], in_=a_t[ch * 128:(ch + 1) * 128, :])
                at.append(t)
                t = cpool.tile([128, DOUT], BF16, tag=f"w2{ch}")
                nc.gpsimd.dma_start(out=t[:, :], in_=w2[ch * 128:(ch + 1) * 128, :])
                w2t.append(t)

            # Tiny warmup gelu: pays the ~2.7us ACT table load during the
            # pipeline ramp instead of on the first real gelu.
            scratch = cpool.tile([128, 2], F32, tag="scratch")
            nc.vector.memset(scratch[:, :], 0.0)
            nc.scalar.activation(scratch[:, :], scratch[:, :], gelu)

            drain_idx = 0
            for g in range(NG):
                # h_buf / g_buf free layout: (r, ch, o) -> (r*2+ch)*256 + o
                h_buf = hpool.tile([128, R * 512], BF16, tag="h")   # [128, 4096]
                g_buf = gpool.tile([128, R * 512], BF16, tag="g")

                for r in range(R):
                    row = g * R + r
                    for ch in range(2):
                        s = (r * 2 + ch) * 256
                        # bf16 in + bf16 out lets the DVE pick the 2x packed
                        # perf mode (fp32 anywhere in the stream forces 1x).
                        nc.vector.tensor_scalar_add(
                            out=h_buf[:, s:s + 256],
                            in0=ct[ch][:, :],
                            scalar1=at[ch][:, row:row + 1],
                        )
                # Two half-group gelus (FD=2048 each): fine enough for the
                # downstream to start early, coarse enough to amortize the
                # ~224-cycle ACTIVATE overhead.
                nc.scalar.activation(
                    g_buf[:, 0:R * 256], h_buf[:, 0:R * 256], gelu)
                nc.scalar.activation(
                    g_buf[:, R * 256:R * 512], h_buf[:, R * 256:R * 512], gelu)

                ostage = opool.tile([128, R * 512], F32, tag="ostage")  # (r, c, d)

                for rr in range(R // 2):   # 2 rows per psum tile (2 banks)
                    ps = pspool.tile([128, 1024], F32, tag="ps")
                    for r2 in range(2):
                        rl = rr * 2 + r2           # row within group
                        for c in range(2):         # o chunk
                            for ch in range(2):    # dh chunk (contraction)
                                nc.tensor.matmul(
                                    out=ps[:, r2 * 512 + c * 256: r2 * 512 + (c + 1) * 256],
                                    lhsT=g_buf[:, (rl * 2 + ch) * 256 + c * 128:
                                               (rl * 2 + ch) * 256 + c * 128 + 128],
                                    rhs=w2t[ch][:, :],
                                    start=(ch == 0),
                                    stop=(ch == 1),
                                )
                    dst = ostage[:, rr * 1024:(rr + 1) * 1024]
                    # Balance PSUM drains: ACT ~31/64 (ACT also does the gelu,
                    # DVE also does the adds).
                    if (drain_idx * 31) % 64 < 31:
                        nc.scalar.copy(dst, ps[:, :])
                    else:
                        nc.vector.tensor_copy(dst, ps[:, :])
                    drain_idx += 1

                    if g == 0 or g == NG - 1:
                        # First/last group: store each drained 512 KiB slice
                        # immediately so the output DMA ramps up early and the
                        # tail after the final drain is short.
                        dma_eng = nc.sync if drain_idx % 2 == 0 else nc.gpsimd
                        dma_eng.dma_start(
                            out=out[g][:, rr * 1024:(rr + 1) * 1024], in_=dst
                        )

                if 0 < g < NG - 1:
                    # Middle groups: one big 2 MiB store. Alternate between the
                    # SP HWDGE ring and the GPSIMD SWDGE ring — both engines
                    # are otherwise idle, so a store blocked on its drain sem
                    # never stalls compute (ACT's ring is strict FIFO: a store
                    # there blocks subsequent gelus).
                    dma_eng = nc.sync if g % 2 == 0 else nc.gpsimd
                    dma_eng.dma_start(out=out[g], in_=ostage[:, :])

    nc.finalize()
    return nc


def _get_nc():
    global _nc_cache
    if _nc_cache is None:
        _nc_cache = build_nc()
    return _nc_cache


def make_in_maps(x, query, W_pre, b_pre, W_emb, b_emb, W1, b1, W2, b2):
    x = np.asarray(x, np.float32)
    query = np.asarray(query, np.float32)
    W_pre = np.asarray(W_pre, np.float32)
    b_pre = np.asarray(b_pre, np.float32)
    W_emb = np.asarray(W_emb, np.float32)
    b_emb = np.asarray(b_emb, np.float32)
    W1 = np.asarray(W1, np.float32)
    b1 = np.asarray(b1, np.float32)
    W2 = np.asarray(W2, np.float32)

    xp = x.reshape(B * NI, DIN) @ W_pre + b_pre
    A = xp @ W1[:DH] + b1                       # [B*NI, DH]
    c = query.reshape(B * NO, DQ) @ W_emb + b_emb
    C2 = c @ W1[DH:]                            # [B*NO, DH]
    A = A.reshape(B, NI, DH)
    C2 = C2.reshape(B, NO, DH)

    w2b = np.ascontiguousarray(W2.astype(ml_dtypes.bfloat16))
    in_maps = []
    for k in range(NCORES):
        b = k // 2
        h = k % 2
        in_maps.append({
            "c_t": np.ascontiguousarray(C2[b].T.astype(ml_dtypes.bfloat16)),
            "a_t": np.ascontiguousarray(A[b, h * 128:(h + 1) * 128, :].T),
            "w2": w2b,
        })
    return in_maps


def run_on_device(in_maps, trace=False):
    nc = _get_nc()
    return run_bass_kernel_spmd(nc, in_maps, core_ids=list(range(NCORES)), trace=trace)


def assemble(results, b2):
    out = np.empty((B, NI, NO, DOUT), np.float32)
    for k in range(NCORES):
        b = k // 2
        h = k % 2
        # dev out: [NG, p, (r, c, d)] with o = c*128 + p, i = g*R + r
        dev = results[k]["out"].reshape(NG, 128, R, 2, DOUT)
        out[b, h * 128:(h + 1) * 128] = (
            dev.transpose(0, 2, 3, 1, 4).reshape(RPC, NO, DOUT)
        )
    b2 = np.asarray(b2, np.float32)
    if np.any(b2):
        out += b2
    return out


def kernel(x, query, W_pre, b_pre, W_emb, b_emb, W1, b1, W2, b2):
    in_maps = make_in_maps(x, query, W_pre, b_pre, W_emb, b_emb, W1, b1, W2, b2)
    res = run_on_device(in_maps, trace=False)
    return assemble(res.results, b2)


# revision 46
# speedup vs baseline: 1.0689x; 1.0689x over previous
"""CondMlp Trainium2 kernel.

Math (reference):
    xp = x @ W_pre + b_pre                 # [B, NI, DH]
    c  = query @ W_emb + b_emb             # [B, NO, DH]
    A  = xp @ W1[:DH] + b1                 # [B, NI, DH]   (host precompute, tiny)
    C2 = c @ W1[DH:]                       # [B, NO, DH]   (host precompute, tiny)
    h[b,i,o,:] = A[b,i,:] + C2[b,o,:]
    out[b,i,o,:] = gelu(h) @ W2 + b2       # [B, NI, NO, DOUT]

The heavy part (device): for every (b,i) row, gelu of a [NO, DH] tile followed
by a [NO, DH] @ [DH, DOUT] matmul, and 256 MB of output writes.

Sharding: 8 cores, core k handles batch b = k//2, NI-half h = k%2 (128 rows).

Device layout trick: work in transposed [dh, o] layout so the broadcast add
A[b,i,:] + C2[b,o,:] becomes a per-partition scalar add (DVE tensor_scalar),
and the second matmul uses G.T slices as the stationary operand:
    psum[o_chunk, dout] += G.T[dh_chunk, o_chunk].T @ W2[dh_chunk, :]
which yields output tiles in [o, dout] layout = contiguous HBM rows.
"""

import numpy as np
import ml_dtypes

import concourse.bass as bass
import concourse.bacc as bacc
import concourse.mybir as mybir
from concourse.tile import TileContext
from concourse.bass_utils import run_bass_kernel_spmd

B, NI, NO = 4, 256, 256
DIN, DQ, DH, DOUT = 256, 256, 256, 256
NCORES = 8
RPC = (B * NI) // NCORES    # rows per core = 128
R = 8                       # rows per group
NG = RPC // R               # 16 groups
F32 = mybir.dt.float32
BF16 = mybir.dt.bfloat16

_nc_cache = None


def build_nc():
    # Bacc (not raw Bass): its finalize() runs generate_event_semaphores,
    # which splits multi-sem waits to satisfy the 1-wait-per-instruction
    # TPB ISA constraint (walrus "Too many sync wait commands" otherwise).
    nc = bacc.Bacc()

    # C2.T in bf16: it is the streamed operand of the DVE tensor_scalar adds,
    # and 16-bit streams hit the 2x packed perf mode (fp32 streams at 1x).
    # A.T stays fp32 (bass requires a float32 scalar operand); the add's fp32
    # output keeps the gelu input near-exact.
    c_t = nc.declare_dram_parameter("c_t", [DH, NO], BF16, isOutput=False)
    a_t = nc.declare_dram_parameter("a_t", [DH, RPC], F32, isOutput=False)
    w2 = nc.declare_dram_parameter("w2", [DH, DOUT], BF16, isOutput=False)       # [dh, dout]
    # Flat per-group output: OUT[g, p, (r, c, d)]; host reassembles. Gives the
    # store DMA 16 KiB contiguous runs per partition instead of 1 KiB.
    out = nc.declare_dram_parameter("out", [NG, 128, R * 512], F32, isOutput=True)

    gelu = mybir.ActivationFunctionType.Gelu

    with TileContext(nc) as tc:
        with (
            tc.tile_pool(name="const", bufs=1) as cpool,
            tc.tile_pool(name="h", bufs=3) as hpool,
            tc.tile_pool(name="g", bufs=3) as gpool,
            tc.tile_pool(name="ps", bufs=4, space="PSUM") as pspool,
            tc.tile_pool(name="ostage", bufs=3) as opool,
        ):
            # Constant tiles, loaded once. dh is split into two partition chunks.
            ct = []
            at = []
            w2t = []
            # Spread the six constant loads over two DMA rings so they land in
            # ~2us instead of ~6 serialized (they gate the whole ramp).
            for ch in range(2):
                t = cpool.tile([128, NO], BF16, tag=f"ct{ch}")
                nc.sync.dma_start(out=t[:, :], in_=c_t[ch * 128:(ch + 1) * 128, :])
                ct.append(t)
                t = cpool.tile([128, RPC], F32, tag=f"at{ch}")
                nc.gpsimd.dma_start(out=t[:, :], in_=a_t[ch * 128:(ch + 1) * 128, :])
                at.append(t)
                t = cpool.tile([128, DOUT], BF16, tag=f"w2{ch}")
                nc.gpsimd.dma_start(out=t[:, :], in_=w2[ch * 128:(ch + 1) * 128, :])
                w2t.append(t)

            # Tiny warmup gelu: pays the ~2.7us ACT table load during the
            # pipeline ramp instead of on the first real gelu.
            scratch = cpool.tile([128, 2], F32, tag="scratch")
            nc.vector.memset(scratch[:, :], 0.0)
            nc.scalar.activation(scratch[:, :], scratch[:, :], gelu)

            drain_idx = 0
            for g in range(NG):
                # h_buf / g_buf free layout: (r, ch, o) -> (r*2+ch)*256 + o
                h_buf = hpool.tile([128, R * 512], BF16, tag="h")   # [128, 4096]
                g_buf = gpool.tile([128, R * 512], BF16, tag="g")

                for r in range(R):
                    row = g * R + r
                    for ch in range(2):
                        s = (r * 2 + ch) * 256
                        # bf16 in + bf16 out lets the DVE pick the 2x packed
                        # perf mode (fp32 anywhere in the stream forces 1x).
                        nc.vector.tensor_scalar_add(
                            out=h_buf[:, s:s + 256],
                            in0=ct[ch][:, :],
                            scalar1=at[ch][:, row:row + 1],
                        )
                # Two half-group gelus (FD=2048 each): fine enough for the
                # downstream to start early, coarse enough to amortize the
                # ~224-cycle ACTIVATE overhead.
                nc.scalar.activation(
                    g_buf[:, 0:R * 256], h_buf[:, 0:R * 256], gelu)
                nc.scalar.activation(
                    g_buf[:, R * 256:R * 512], h_buf[:, R * 256:R * 512], gelu)

                ostage = opool.tile([128, R * 512], F32, tag="ostage")  # (r, c, d)

                for rr in range(R // 2):   # 2 rows per psum tile (2 banks)
                    ps = pspool.tile([128, 1024], F32, tag="ps")
                    for r2 in range(2):
                        rl = rr * 2 + r2           # row within group
                        for c in range(2):         # o chunk
                            for ch in range(2):    # dh chunk (contraction)
                                nc.tensor.matmul(
                                    out=ps[:, r2 * 512 + c * 256: r2 * 512 + (c + 1) * 256],
                                    lhsT=g_buf[:, (rl * 2 + ch) * 256 + c * 128:
                                               (rl * 2 + ch) * 256 + c * 128 + 128],
                                    rhs=w2t[ch][:, :],
                                    start=(ch == 0),
                                    stop=(ch == 1),
                                )
                    dst = ostage[:, rr * 1024:(rr + 1) * 1024]
                    # Balance PSUM drains: ACT ~31/64 (ACT also does the gelu,
                    # DVE also does the adds).
                    if (drain_idx * 31) % 64 < 31:
                        nc.scalar.copy(dst, ps[:, :])
                    else:
                        nc.vector.tensor_copy(dst, ps[:, :])
                    drain_idx += 1

                    if g == 0 or g == NG - 1:
                        # First/last group: store each drained 512 KiB slice
                        # immediately so the output DMA ramps up early and the
                        # tail after the final drain is short.
                        dma_eng = nc.sync if drain_idx % 2 == 0 else nc.gpsimd
                        dma_eng.dma_start(
                            out=out[g][:, rr * 1024:(rr + 1) * 1024], in_=dst
                        )

                if 0 < g < NG - 1:
                    # Middle groups: one big 2 MiB store. Alternate between the
                    # SP HWDGE ring and the GPSIMD SWDGE ring — both engines
                    # are otherwise idle, so a store blocked on its drain sem
                    # never stalls compute (ACT's ring is strict FIFO: a store
                    # there blocks subsequent gelus).
                    dma_eng = nc.sync if g % 2 == 0 else nc.gpsimd
                    dma_eng.dma_start(out=out[g], in_=ostage[:, :])

    nc.finalize()
    return nc


def _get_nc():
    global _nc_cache
    if _nc_cache is None:
        _nc_cache = build_nc()
    return _nc_cache


def make_in_maps(x, query, W_pre, b_pre, W_emb, b_emb, W1, b1, W2, b2):
    x = np.asarray(x, np.float32)
    query = np.asarray(query, np.float32)
    W_pre = np.asarray(W_pre, np.float32)
    b_pre = np.asarray(b_pre, np.float32)
    W_emb = np.asarray(W_emb, np.float32)
    b_emb = np.asarray(b_emb, np.float32)
    W1 = np.asarray(W1, np.float32)
    b1 = np.asarray(b1, np.float32)
    W2 = np.asarray(W2, np.float32)

    xp = x.reshape(B * NI, DIN) @ W_pre + b_pre
    A = xp @ W1[:DH] + b1                       # [B*NI, DH]
    c = query.reshape(B * NO, DQ) @ W_emb + b_emb
    C2 = c @ W1[DH:]                            # [B*NO, DH]
    A = A.reshape(B, NI, DH)
    C2 = C2.reshape(B, NO, DH)

    w2b = np.ascontiguousarray(W2.astype(ml_dtypes.bfloat16))
    in_maps = []
    for k in range(NCORES):
        b = k // 2
        h = k % 2
        in_maps.append({
            "c_t": np.ascontiguousarray(C2[b].T.astype(ml_dtypes.bfloat16)),
            "a_t": np.ascontiguousarray(A[b, h * 128:(h + 1) * 128, :].T),
            "w2": w2b,
        })
    return in_maps


def run_on_device(in_maps, trace=False):
    nc = _get_nc()
    return run_bass_kernel_spmd(nc, in_maps, core_ids=list(range(NCORES)), trace=trace)


def assemble(results, b2):
    out = np.empty((B, NI, NO, DOUT), np.float32)
    for k in range(NCORES):
        b = k // 2
        h = k % 2
        # dev out: [NG, p, (r, c, d)] with o = c*128 + p, i = g*R + r
        dev = results[k]["out"].reshape(NG, 128, R, 2, DOUT)
        out[b, h * 128:(h + 1) * 128] = (
            dev.transpose(0, 2, 3, 1, 4).reshape(RPC, NO, DOUT)
        )
    b2 = np.asarray(b2, np.float32)
    if np.any(b2):
        out += b2
    return out


def kernel(x, query, W_pre, b_pre, W_emb, b_emb, W1, b1, W2, b2):
    in_maps = make_in_maps(x, query, W_pre, b_pre, W_emb, b_emb, W1, b1, W2, b2)
    res = run_on_device(in_maps, trace=False)
    return assemble(res.results, b2)
